# revision 18
# baseline (speedup 1.0000x reference)
"""Trainium2 Bass kernel for nn_BackupBarrierCBF.

Reference semantics (B=1024, A=64, T=50 unicycle rollout + rect-vs-disc
distance + min-over-horizon + saturation). Crucial subtleties:
  - braking controller: u = (-9*tanh(2*v), 0) => theta is CONSTANT, so
    positions are x0 + cos(theta)*dt*cumsum(v).
  - veh_veh_distance receives traj[..., 0:3] = (x, y, v): the body-frame
    rotation angle is the (time-varying) VELOCITY, not theta.
  - traj slot k holds the state AFTER k+1 steps: position cumsum uses
    v_0..v_k while the stored rotation angle is v_{k+1}.

Per-core structure (batch rows on the 128 partitions):
  - 50-step serial v-recurrence (ACT Tanh + DVE scalar_tensor_tensor)
    writing straight into a t-major trajectory (all chain ops contiguous);
    the col-major cumsum ST is built by per-step adds and the angle range
    reduction runs in the rollout's DVE slack. Constants precede the
    rollout so their ACT Sins/Sqrts don't thrash the Tanh table.
  - sin/cos of v(t) on ACT with col-major STRIDED writes (2.2x ACT penalty,
    but ACT has slack and every later DVE op stays unit-stride). Range
    reduction only for the first k_red slots (|v| provably <= pi afterward:
    while |v|>2.2 each step shrinks |v| by >= 0.8997 and the map keeps
    |v| <= pi once below). cos x = sin(pi/2 - |x|).
  - distance phase: ~28 big [128, 64, 50] DVE ops, a-major, unit inner
    stride, per-agent constants broadcast with 0-step APs; SINV-products
    ordered first (COSV finishes later on ACT); abs on ACT, fine-grained.
  - NO gpsimd tensor work: gpsimd shares the DVE SBUF port (measured 2.5x
    DVE slowdown when overlapped - net zero).

Sharding: pure data parallel over batch B across 8 cores (128 rows/core).
"""
import numpy as np
import concourse.bass as bass
import concourse.bacc as bacc
import concourse.tile as tile
from concourse import mybir
from concourse.bass_utils import run_bass_kernel_spmd

F32 = mybir.dt.float32
I32 = mybir.dt.int32
OP = mybir.AluOpType
ACT = mybir.ActivationFunctionType

B, A, F = 1024, 64, 15
N_CORES = 8
PB = B // N_CORES          # 128 batch rows per core (partition dim)
T = 50
NC2 = 2 * A                # 128 columns: [ego agents | other agents]
NT = T * A                 # 3200
TWO_PI = float(2.0 * np.pi)

_cache: dict = {}


def _ap(t: bass.AP, extra_offset: int, free_dims: list) -> bass.AP:
    """View into tile t: keep partition dim, replace free dims."""
    return bass.AP(tensor=t.tensor, offset=t.offset + extra_offset,
                   ap=[list(t.ap[0])] + [list(d) for d in free_dims])


def _build(dt_uniform, k_red):
    nc = bacc.Bacc("TRN2", target_bir_lowering=False)
    data = nc.dram_tensor("data", [PB, A * F], F32, kind="ExternalInput")
    out = nc.dram_tensor("out", [PB, A], F32, kind="ExternalOutput")

    with tile.TileContext(nc) as tc:
        with tc.tile_pool(name="pool", bufs=1) as pool:
            # ---------------- load ----------------
            D = pool.tile([PB, A * F], F32)
            half = (A * F) // 2
            nc.sync.dma_start(out=_ap(D, 0, [[1, half]]),
                              in_=_ap(data.ap(), 0, [[1, half]]))
            nc.sync.dma_start(out=_ap(D, half, [[1, A * F - half]]),
                              in_=_ap(data.ap(), half, [[1, A * F - half]]))

            def fld(k):  # [128, 64] strided view of per-agent field k
                return _ap(D, k, [[F, A]])

            halfpi = pool.tile([PB, 1], F32)
            nc.vector.memset(halfpi[:], float(np.pi / 2))

            cons = pool.tile([PB, 12, A], F32)

            def c(i):
                return _ap(cons, i * A, [[1, A]])

            def cb(i):  # broadcast over inner t: [128, 64, T]
                return _ap(cons, i * A, [[1, A], [0, T]])

            C_P0X, C_P0Y = 0, 1
            C_D1, C_D2, C_D3, C_K2Y = 2, 3, 4, 5
            C_CEDT, C_SEDT, C_CADT, C_SADT = 6, 7, 8, 9
            C_RE, C_RA = 10, 11

            scr = pool.tile([PB, 10, A], F32)

            def s(i):
                return _ap(scr, i * A, [[1, A]])

            ki = pool.tile([PB, 4, A], I32)

            def kis(i):
                return _ap(ki, i * A, [[1, A]])

            # ---------------- per-agent constants (front) ------------
            # Their ACT Sins/Sqrts run before any Tanh so the ACT table is
            # loaded once per function; four separate scratches keep the
            # sincos pipelines independent.
            def sincos(theta_ap, out_sin, out_cos, base):
                for idx, (want_cos, dst) in enumerate(((False, out_sin),
                                                       (True, out_cos))):
                    sc = s(base + idx)
                    shift = 0.25 if want_cos else 0.0
                    nc.vector.tensor_scalar(out=sc, in0=theta_ap,
                                            scalar1=1.0 / TWO_PI, scalar2=shift,
                                            op0=OP.mult, op1=OP.add)
                    nc.vector.tensor_copy(out=kis(base + idx), in_=sc)
                    nc.vector.tensor_copy(out=sc, in_=kis(base + idx))
                    nc.vector.scalar_tensor_tensor(
                        out=sc, in0=sc, scalar=-TWO_PI, in1=theta_ap,
                        op0=OP.mult, op1=OP.add)
                    nc.scalar.activation(
                        out=dst, in_=sc, func=ACT.Sin,
                        bias=halfpi[:] if want_cos else 0.0, scale=1.0)

            sincos(fld(7), c(C_SADT), c(C_CADT), 0)
            sincos(fld(3), c(C_SEDT), c(C_CEDT), 2)
            for i in (C_CADT, C_SADT, C_CEDT, C_SEDT):
                nc.vector.tensor_mul(out=c(i), in0=c(i), in1=fld(14))

            nc.vector.tensor_mul(out=s(4), in0=fld(8), in1=fld(8))
            nc.vector.tensor_mul(out=s(5), in0=fld(9), in1=fld(9))
            nc.vector.tensor_add(out=s(4), in0=s(4), in1=s(5))
            nc.scalar.activation(out=c(C_RE), in_=s(4), func=ACT.Sqrt,
                                 scale=0.25)
            nc.vector.tensor_mul(out=s(6), in0=fld(11), in1=fld(11))
            nc.vector.tensor_mul(out=s(7), in0=fld(12), in1=fld(12))
            nc.vector.tensor_add(out=s(6), in0=s(6), in1=s(7))
            nc.scalar.activation(out=c(C_RA), in_=s(6), func=ACT.Sqrt,
                                 scale=0.25)
            # d1 = 0.5*(We-Le); d2 = 0.5*(Wa-La); k1y = 0.5*We+ra;
            # k2y = 0.5*Wa+re; d3 = k2y-k1y
            nc.vector.tensor_sub(out=s(8), in0=fld(9), in1=fld(8))
            nc.vector.tensor_scalar_mul(out=c(C_D1), in0=s(8), scalar1=0.5)
            nc.vector.tensor_sub(out=s(9), in0=fld(12), in1=fld(11))
            nc.vector.tensor_scalar_mul(out=c(C_D2), in0=s(9), scalar1=0.5)
            nc.vector.scalar_tensor_tensor(
                out=s(8), in0=fld(9), scalar=0.5, in1=c(C_RA),
                op0=OP.mult, op1=OP.add)          # k1y
            nc.vector.scalar_tensor_tensor(
                out=c(C_K2Y), in0=fld(12), scalar=0.5, in1=c(C_RE),
                op0=OP.mult, op1=OP.add)          # k2y
            nc.vector.tensor_sub(out=c(C_D3), in0=c(C_K2Y), in1=s(8))
            nc.vector.tensor_sub(out=c(C_P0X), in0=fld(4), in1=fld(0))
            nc.vector.tensor_sub(out=c(C_P0Y), in0=fld(5), in1=fld(1))

            # ---------------- rollout ----------------
            # Serial chain writes straight into t-major VT (slot j at
            # j*NC2); col-major ST built by per-step adds; the angle
            # range-reduce fills the rollout's DVE slack.
            VT = pool.tile([PB, (T + 1) * NC2], F32, tag="tVT")
            ST = pool.tile([PB, NC2 * T], F32, tag="tST")

            def vslot(j):  # j=0: strided input view; j in 1..50: contiguous
                if j == 0:
                    return _ap(D, 2, [[4, 2], [F, A]])
                return _ap(VT, j * NC2, [[1, NC2]])

            def stslot(k):  # k in 0..49, col-major strided
                return _ap(ST, k, [[T, NC2]])

            G = pool.tile([PB, NC2], F32)
            nc.vector.tensor_copy(out=stslot(0), in_=vslot(0))

            if dt_uniform is None:
                NDT2 = pool.tile([PB, NC2], F32)
                nc.vector.tensor_scalar_mul(
                    out=NDT2[:], in0=_ap(D, 14, [[0, 2], [F, A]]), scalar1=-9.0)

            MS = KI2 = None
            if k_red > 0:
                MS = pool.tile([PB, NC2 * k_red], F32, tag="tPXY")
                KI2 = pool.tile([PB, NC2 * k_red], I32, tag="tSCR")

            SEv = _ap(ST, 0, [[T, A], [1, T]])
            SAv = _ap(ST, A * T, [[T, A], [1, T]])
            PXY = pool.tile([PB, 2 * NT], F32, tag="tPXY")
            SCR = pool.tile([PB, 2 * NT], F32, tag="tSCR")

            for j in range(1, T + 1):
                nc.scalar.activation(out=G[:], in_=vslot(j - 1),
                                     func=ACT.Tanh, scale=2.0)
                if dt_uniform is None:
                    nc.vector.tensor_mul(out=G[:], in0=G[:], in1=NDT2[:])
                    nc.vector.tensor_add(out=vslot(j), in0=vslot(j - 1),
                                         in1=G[:])
                else:
                    nc.vector.scalar_tensor_tensor(
                        out=vslot(j), in0=G[:], scalar=-9.0 * float(dt_uniform),
                        in1=vslot(j - 1), op0=OP.mult, op1=OP.add)
                if j < T:
                    nc.vector.tensor_add(out=stslot(j), in0=stslot(j - 1),
                                         in1=vslot(j))
                if j == k_red and k_red > 0:
                    # range-reduce angle slots 1..k_red in place (all
                    # ST-adds reading the raw values already emitted)
                    red_view = _ap(VT, NC2, [[1, NC2 * k_red]])
                    nc.vector.tensor_scalar_mul(out=MS[:], in0=red_view,
                                                scalar1=1.0 / TWO_PI)
                    nc.vector.tensor_copy(out=KI2[:], in_=MS[:])
                    nc.vector.tensor_copy(out=MS[:], in_=KI2[:])
                    nc.vector.scalar_tensor_tensor(
                        out=red_view, in0=MS[:], scalar=-TWO_PI, in1=red_view,
                        op0=OP.mult, op1=OP.add)

            # ---------------- trig of v (angles are v_{k+1}) ----------
            # t-major contiguous reads, col-major strided writes (ACT has
            # slack; DVE consumers stay unit-stride).  SINV first so the
            # rel phase's SINV-products can start earliest.
            ang = _ap(VT, NC2, [[1, T * NC2]])
            SINV = pool.tile([PB, NC2 * T], F32)
            COSV = pool.tile([PB, NC2 * T], F32)
            cm_out_sin = _ap(SINV, 0, [[1, T], [T, NC2]])
            cm_out_cos = _ap(COSV, 0, [[1, T], [T, NC2]])
            nc.scalar.activation(out=cm_out_sin, in_=ang, func=ACT.Sin)
            nc.scalar.activation(out=cm_out_cos, in_=ang, func=ACT.Abs)
            nc.scalar.activation(out=COSV[:], in_=COSV[:], func=ACT.Sin,
                                 bias=halfpi[:], scale=-1.0)

            S1 = _ap(SCR, 0, [[1, NT]])
            S2 = _ap(SCR, NT, [[1, NT]])
            PX = _ap(PXY, 0, [[1, NT]])
            PY = _ap(PXY, NT, [[1, NT]])

            nc.vector.tensor_mul(out=S1, in0=SAv, in1=cb(C_CADT))
            nc.vector.tensor_add(out=S1, in0=S1, in1=cb(C_P0X))
            nc.vector.tensor_mul(out=S2, in0=SEv, in1=cb(C_CEDT))
            nc.vector.tensor_sub(out=PX, in0=S1, in1=S2)
            nc.vector.tensor_mul(out=S1, in0=SAv, in1=cb(C_SADT))
            nc.vector.tensor_add(out=S1, in0=S1, in1=cb(C_P0Y))
            nc.vector.tensor_mul(out=S2, in0=SEv, in1=cb(C_SEDT))
            nc.vector.tensor_sub(out=PY, in0=S1, in1=S2)

            # ---------------- body-frame components ----------------
            # SINV-products first (COSV lands later on ACT).
            CE = _ap(COSV, 0, [[1, NT]])
            CA = _ap(COSV, NT, [[1, NT]])
            SE_ = _ap(SINV, 0, [[1, NT]])
            SA_ = _ap(SINV, NT, [[1, NT]])
            R12 = pool.tile([PB, 2 * NT], F32, tag="tST")
            R1X = _ap(R12, 0, [[1, NT]])
            R1Y = _ap(R12, NT, [[1, NT]])
            R34 = pool.tile([PB, 2 * NT], F32)
            R2X = _ap(R34, 0, [[1, NT]])
            R2Y = _ap(R34, NT, [[1, NT]])

            nc.vector.tensor_mul(out=R1X, in0=SE_, in1=PY)
            nc.vector.tensor_mul(out=R1Y, in0=SE_, in1=PX)
            nc.vector.tensor_mul(out=R2X, in0=SA_, in1=PY)
            nc.vector.tensor_mul(out=R2Y, in0=SA_, in1=PX)
            nc.vector.tensor_mul(out=S1, in0=CE, in1=PX)
            nc.vector.tensor_add(out=R1X, in0=R1X, in1=S1)   # rel1x
            nc.vector.tensor_mul(out=S2, in0=CE, in1=PY)
            nc.vector.tensor_sub(out=R1Y, in0=S2, in1=R1Y)   # rel1y
            nc.vector.tensor_mul(out=S1, in0=CA, in1=PX)
            nc.vector.tensor_add(out=R2X, in0=R2X, in1=S1)   # -rel2x; |.| ok
            nc.vector.tensor_mul(out=S2, in0=CA, in1=PY)
            nc.vector.tensor_sub(out=R2Y, in0=R2Y, in1=S2)   # rel2y

            # |rel| on ACT, then the shifted max-tree:
            # dist = max(max(|r1x|+d1, |r1y|) + d3, max(|r2x|+d2, |r2y|)) - k2y
            # with d1=k1y-k1x, d2=k2y-k2x, d3=k2y-k1y; -k2y lands after the
            # min-reduce as a [128,64] op (k's are constant over t).
            for R in (R1X, R1Y, R2X, R2Y):
                nc.scalar.activation(out=R, in_=R, func=ACT.Abs)
            nc.vector.tensor_add(out=R1X, in0=R1X, in1=cb(C_D1))
            nc.vector.tensor_tensor(out=R1X, in0=R1X, in1=R1Y, op=OP.max)
            nc.vector.tensor_add(out=R2X, in0=R2X, in1=cb(C_D2))
            nc.vector.tensor_tensor(out=R2X, in0=R2X, in1=R2Y, op=OP.max)
            nc.vector.tensor_add(out=R1X, in0=R1X, in1=cb(C_D3))
            nc.vector.tensor_tensor(out=R1X, in0=R1X, in1=R2X, op=OP.max)

            H = pool.tile([PB, A], F32)
            nc.vector.tensor_reduce(out=H[:],
                                    in_=_ap(R12, 0, [[T, A], [1, T]]),
                                    axis=mybir.AxisListType.X, op=OP.min)
            nc.vector.tensor_sub(out=H[:], in0=H[:], in1=c(C_K2Y))
            OUTT = pool.tile([PB, A], F32)
            nc.scalar.activation(out=H[:], in_=H[:], func=ACT.Tanh, scale=0.1)
            nc.vector.tensor_scalar_mul(out=OUTT[:], in0=H[:], scalar1=5.0)
            nc.sync.dma_start(out=out[:], in_=OUTT[:])

    nc.compile()
    return nc


def _get_nc(dt_uniform, k_red):
    key = ("nc", dt_uniform, k_red)
    if key not in _cache:
        _cache[key] = _build(dt_uniform, k_red)
    return _cache[key]


def _make_runner(nc):
    """One-time build of a cached jitted SPMD executable for nc (the
    equivalent of bass2jax.run_bass_via_pjrt, but reusable across calls so
    repeated kernel() invocations skip the jax retrace)."""
    import jax
    from jax.sharding import Mesh, PartitionSpec
    from jax.experimental.shard_map import shard_map
    from concourse import bass2jax, mybir as _mybir

    bass2jax.install_neuronx_cc_hook()
    partition_name = (nc.partition_id_tensor.name
                      if nc.partition_id_tensor else None)
    in_names, out_names, out_avals, zero_outs = [], [], [], []
    for alloc in nc.m.functions[0].allocations:
        if not isinstance(alloc, _mybir.MemoryLocationSet):
            continue
        name = alloc.memorylocations[0].name
        if alloc.kind == "ExternalInput":
            if name != partition_name:
                in_names.append(name)
        elif alloc.kind == "ExternalOutput":
            shape = tuple(alloc.tensor_shape)
            dtype = _mybir.dt.np(alloc.dtype)
            out_names.append(name)
            out_avals.append(jax.core.ShapedArray(shape, dtype))
            zero_outs.append(np.zeros(shape, dtype))
    n_params = len(in_names)
    all_names = in_names + out_names
    if partition_name is not None:
        all_names = all_names + [partition_name]
    donate = tuple(range(n_params, n_params + len(out_names)))

    def _body(*args):
        operands = list(args)
        if partition_name is not None:
            operands.append(bass2jax.partition_id_tensor())
        outs = bass2jax._bass_exec_p.bind(
            *operands, out_avals=tuple(out_avals), in_names=tuple(all_names),
            out_names=tuple(out_names), lowering_input_output_aliases=(),
            sim_require_finite=True, sim_require_nnan=True, nc=nc)
        return tuple(outs)

    mesh = Mesh(np.asarray(jax.devices()[:N_CORES]), ("core",))
    in_specs = (PartitionSpec("core"),) * (n_params + len(out_names))
    out_specs = (PartitionSpec("core"),) * len(out_names)
    sharded = jax.jit(
        shard_map(_body, mesh=mesh, in_specs=in_specs, out_specs=out_specs,
                  check_rep=False),
        donate_argnums=donate, keep_unused=True)
    concat_zeros = [np.zeros((N_CORES * z.shape[0], *z.shape[1:]), z.dtype)
                    for z in zero_outs]

    def run(full_data_2d):  # [B, A*F] -> [B, A]
        outs = sharded(full_data_2d, *[z.copy() for z in concat_zeros])
        return np.asarray(outs[out_names.index("out")])

    return run


def _params_for(data: np.ndarray):
    dt = data[..., 14]
    dt0 = float(dt.flat[0])
    dt_uniform = dt0 if bool(np.all(dt == dt0)) else None
    vmax = float(np.abs(data[..., [2, 6]]).max())
    # slots j >= k_red have |v_j| <= pi: while |v| > 2.2 each step shrinks
    # |v| by >= 9*dt_min*tanh(4.4), and the map keeps |v| <= pi once below
    # (valid when the max step 9*dt_max <= pi; otherwise reduce every slot).
    dt_min = float(dt.min())
    dt_max = float(dt.max())
    shrink = 9.0 * dt_min * 0.9997
    if 9.0 * dt_max > np.pi or shrink <= 1e-6:
        k_red = T
    else:
        k_red = int(min(T, max(0, np.ceil((vmax - np.pi) / shrink) + 1)))
    return dt_uniform, k_red


def _run(data: np.ndarray, trace: bool = False):
    data = np.ascontiguousarray(data, dtype=np.float32)
    assert data.shape == (B, A, F), data.shape
    dt_uniform, k_red = _params_for(data)
    nc = _get_nc(dt_uniform, k_red)
    in_maps = [{"data": data[c * PB:(c + 1) * PB].reshape(PB, A * F)}
               for c in range(N_CORES)]
    res = run_bass_kernel_spmd(nc, in_maps, core_ids=list(range(N_CORES)),
                               trace=trace)
    full = np.concatenate([res.results[c]["out"] for c in range(N_CORES)],
                          axis=0)
    return full, res


def kernel(data: np.ndarray) -> np.ndarray:
    data = np.ascontiguousarray(data, dtype=np.float32)
    assert data.shape == (B, A, F), data.shape
    dt_uniform, k_red = _params_for(data)
    key = ("runner", dt_uniform, k_red)
    if key not in _cache:
        _cache[key] = _make_runner(_get_nc(dt_uniform, k_red))
    return _cache[key](data.reshape(B, A * F)).astype(np.float32)


# revision 19
# speedup vs baseline: 1.0014x; 1.0014x over previous
"""Trainium2 Bass kernel for nn_BackupBarrierCBF.

Reference semantics (B=1024, A=64, T=50 unicycle rollout + rect-vs-disc
distance + min-over-horizon + saturation). Crucial subtleties:
  - braking controller: u = (-9*tanh(2*v), 0) => theta is CONSTANT, so
    positions are x0 + cos(theta)*dt*cumsum(v).
  - veh_veh_distance receives traj[..., 0:3] = (x, y, v): the body-frame
    rotation angle is the (time-varying) VELOCITY, not theta.
  - traj slot k holds the state AFTER k+1 steps: position cumsum uses
    v_0..v_k while the stored rotation angle is v_{k+1}.

Per-core structure (batch rows on the 128 partitions):
  - 50-step serial v-recurrence (ACT Tanh + DVE scalar_tensor_tensor)
    writing straight into a t-major trajectory (all chain ops contiguous);
    the col-major cumsum ST is built by per-step adds and the angle range
    reduction runs in the rollout's DVE slack. Constants precede the
    rollout so their ACT Sins/Sqrts don't thrash the Tanh table.
  - sin/cos of v(t) on ACT with col-major STRIDED writes (2.2x ACT penalty,
    but ACT has slack and every later DVE op stays unit-stride). Range
    reduction only for the first k_red slots (|v| provably <= pi afterward:
    while |v|>2.2 each step shrinks |v| by >= 0.8997 and the map keeps
    |v| <= pi once below). cos x = sin(pi/2 - |x|).
  - distance phase: ~28 big [128, 64, 50] DVE ops, a-major, unit inner
    stride, per-agent constants broadcast with 0-step APs; SINV-products
    ordered first (COSV finishes later on ACT); abs on ACT, fine-grained.
  - NO gpsimd tensor work: gpsimd shares the DVE SBUF port (measured 2.5x
    DVE slowdown when overlapped - net zero).

Sharding: pure data parallel over batch B across 8 cores (128 rows/core).
"""
import numpy as np
import concourse.bass as bass
import concourse.bacc as bacc
import concourse.tile as tile
from concourse import mybir
from concourse.bass_utils import run_bass_kernel_spmd

F32 = mybir.dt.float32
I32 = mybir.dt.int32
OP = mybir.AluOpType
ACT = mybir.ActivationFunctionType

B, A, F = 1024, 64, 15
N_CORES = 8
PB = B // N_CORES          # 128 batch rows per core (partition dim)
T = 50
NC2 = 2 * A                # 128 columns: [ego agents | other agents]
NT = T * A                 # 3200
TWO_PI = float(2.0 * np.pi)

_cache: dict = {}


def _ap(t: bass.AP, extra_offset: int, free_dims: list) -> bass.AP:
    """View into tile t: keep partition dim, replace free dims."""
    return bass.AP(tensor=t.tensor, offset=t.offset + extra_offset,
                   ap=[list(t.ap[0])] + [list(d) for d in free_dims])


def _build(dt_uniform, k_red):
    nc = bacc.Bacc("TRN2", target_bir_lowering=False)
    data = nc.dram_tensor("data", [PB, A * F], F32, kind="ExternalInput")
    out = nc.dram_tensor("out", [PB, A], F32, kind="ExternalOutput")

    with tile.TileContext(nc) as tc:
        with tc.tile_pool(name="pool", bufs=1) as pool:
            # ---------------- load ----------------
            D = pool.tile([PB, A * F], F32)
            nc.sync.dma_start(out=D[:], in_=data[:])

            def fld(k):  # [128, 64] strided view of per-agent field k
                return _ap(D, k, [[F, A]])

            halfpi = pool.tile([PB, 1], F32)
            nc.vector.memset(halfpi[:], float(np.pi / 2))

            cons = pool.tile([PB, 12, A], F32)

            def c(i):
                return _ap(cons, i * A, [[1, A]])

            def cb(i):  # broadcast over inner t: [128, 64, T]
                return _ap(cons, i * A, [[1, A], [0, T]])

            C_P0X, C_P0Y = 0, 1
            C_D1, C_D2, C_D3, C_K2Y = 2, 3, 4, 5
            C_CEDT, C_SEDT, C_CADT, C_SADT = 6, 7, 8, 9
            C_RE, C_RA = 10, 11

            scr = pool.tile([PB, 10, A], F32)

            def s(i):
                return _ap(scr, i * A, [[1, A]])

            ki = pool.tile([PB, 4, A], I32)

            def kis(i):
                return _ap(ki, i * A, [[1, A]])

            # ---------------- per-agent constants (front) ------------
            # Their ACT Sins/Sqrts run before any Tanh so the ACT table is
            # loaded once per function; four separate scratches keep the
            # sincos pipelines independent.
            def sincos(theta_ap, out_sin, out_cos, base):
                for idx, (want_cos, dst) in enumerate(((False, out_sin),
                                                       (True, out_cos))):
                    sc = s(base + idx)
                    shift = 0.25 if want_cos else 0.0
                    nc.vector.tensor_scalar(out=sc, in0=theta_ap,
                                            scalar1=1.0 / TWO_PI, scalar2=shift,
                                            op0=OP.mult, op1=OP.add)
                    nc.vector.tensor_copy(out=kis(base + idx), in_=sc)
                    nc.vector.tensor_copy(out=sc, in_=kis(base + idx))
                    nc.vector.scalar_tensor_tensor(
                        out=sc, in0=sc, scalar=-TWO_PI, in1=theta_ap,
                        op0=OP.mult, op1=OP.add)
                    nc.scalar.activation(
                        out=dst, in_=sc, func=ACT.Sin,
                        bias=halfpi[:] if want_cos else 0.0, scale=1.0)

            sincos(fld(7), c(C_SADT), c(C_CADT), 0)
            sincos(fld(3), c(C_SEDT), c(C_CEDT), 2)
            for i in (C_CADT, C_SADT, C_CEDT, C_SEDT):
                nc.vector.tensor_mul(out=c(i), in0=c(i), in1=fld(14))

            nc.vector.tensor_mul(out=s(4), in0=fld(8), in1=fld(8))
            nc.vector.tensor_mul(out=s(5), in0=fld(9), in1=fld(9))
            nc.vector.tensor_add(out=s(4), in0=s(4), in1=s(5))
            nc.scalar.activation(out=c(C_RE), in_=s(4), func=ACT.Sqrt,
                                 scale=0.25)
            nc.vector.tensor_mul(out=s(6), in0=fld(11), in1=fld(11))
            nc.vector.tensor_mul(out=s(7), in0=fld(12), in1=fld(12))
            nc.vector.tensor_add(out=s(6), in0=s(6), in1=s(7))
            nc.scalar.activation(out=c(C_RA), in_=s(6), func=ACT.Sqrt,
                                 scale=0.25)
            # d1 = 0.5*(We-Le); d2 = 0.5*(Wa-La); k1y = 0.5*We+ra;
            # k2y = 0.5*Wa+re; d3 = k2y-k1y
            nc.vector.tensor_sub(out=s(8), in0=fld(9), in1=fld(8))
            nc.vector.tensor_scalar_mul(out=c(C_D1), in0=s(8), scalar1=0.5)
            nc.vector.tensor_sub(out=s(9), in0=fld(12), in1=fld(11))
            nc.vector.tensor_scalar_mul(out=c(C_D2), in0=s(9), scalar1=0.5)
            nc.vector.scalar_tensor_tensor(
                out=s(8), in0=fld(9), scalar=0.5, in1=c(C_RA),
                op0=OP.mult, op1=OP.add)          # k1y
            nc.vector.scalar_tensor_tensor(
                out=c(C_K2Y), in0=fld(12), scalar=0.5, in1=c(C_RE),
                op0=OP.mult, op1=OP.add)          # k2y
            nc.vector.tensor_sub(out=c(C_D3), in0=c(C_K2Y), in1=s(8))
            nc.vector.tensor_sub(out=c(C_P0X), in0=fld(4), in1=fld(0))
            nc.vector.tensor_sub(out=c(C_P0Y), in0=fld(5), in1=fld(1))

            # ---------------- rollout ----------------
            # Serial chain writes straight into t-major VT (slot j at
            # j*NC2); col-major ST built by per-step adds; the angle
            # range-reduce fills the rollout's DVE slack.
            VT = pool.tile([PB, (T + 1) * NC2], F32, tag="tVT")
            ST = pool.tile([PB, NC2 * T], F32, tag="tST")

            def vslot(j):  # j=0: strided input view; j in 1..50: contiguous
                if j == 0:
                    return _ap(D, 2, [[4, 2], [F, A]])
                return _ap(VT, j * NC2, [[1, NC2]])

            def stslot(k):  # k in 0..49, col-major strided
                return _ap(ST, k, [[T, NC2]])

            G = pool.tile([PB, NC2], F32)
            nc.vector.tensor_copy(out=stslot(0), in_=vslot(0))

            if dt_uniform is None:
                NDT2 = pool.tile([PB, NC2], F32)
                nc.vector.tensor_scalar_mul(
                    out=NDT2[:], in0=_ap(D, 14, [[0, 2], [F, A]]), scalar1=-9.0)

            MS = KI2 = None
            if k_red > 0:
                MS = pool.tile([PB, NC2 * k_red], F32, tag="tPXY")
                KI2 = pool.tile([PB, NC2 * k_red], I32, tag="tSCR")

            SEv = _ap(ST, 0, [[T, A], [1, T]])
            SAv = _ap(ST, A * T, [[T, A], [1, T]])
            PXY = pool.tile([PB, 2 * NT], F32, tag="tPXY")
            SCR = pool.tile([PB, 2 * NT], F32, tag="tSCR")

            for j in range(1, T + 1):
                nc.scalar.activation(out=G[:], in_=vslot(j - 1),
                                     func=ACT.Tanh, scale=2.0)
                if dt_uniform is None:
                    nc.vector.tensor_mul(out=G[:], in0=G[:], in1=NDT2[:])
                    nc.vector.tensor_add(out=vslot(j), in0=vslot(j - 1),
                                         in1=G[:])
                else:
                    nc.vector.scalar_tensor_tensor(
                        out=vslot(j), in0=G[:], scalar=-9.0 * float(dt_uniform),
                        in1=vslot(j - 1), op0=OP.mult, op1=OP.add)
                if j < T:
                    nc.vector.tensor_add(out=stslot(j), in0=stslot(j - 1),
                                         in1=vslot(j))
                if j == k_red and k_red > 0:
                    # range-reduce angle slots 1..k_red in place (all
                    # ST-adds reading the raw values already emitted)
                    red_view = _ap(VT, NC2, [[1, NC2 * k_red]])
                    nc.vector.tensor_scalar_mul(out=MS[:], in0=red_view,
                                                scalar1=1.0 / TWO_PI)
                    nc.vector.tensor_copy(out=KI2[:], in_=MS[:])
                    nc.vector.tensor_copy(out=MS[:], in_=KI2[:])
                    nc.vector.scalar_tensor_tensor(
                        out=red_view, in0=MS[:], scalar=-TWO_PI, in1=red_view,
                        op0=OP.mult, op1=OP.add)

            # ---------------- trig of v (angles are v_{k+1}) ----------
            # t-major contiguous reads, col-major strided writes (ACT has
            # slack; DVE consumers stay unit-stride).  SINV first so the
            # rel phase's SINV-products can start earliest.
            ang = _ap(VT, NC2, [[1, T * NC2]])
            SINV = pool.tile([PB, NC2 * T], F32)
            COSV = pool.tile([PB, NC2 * T], F32)
            cm_out_sin = _ap(SINV, 0, [[1, T], [T, NC2]])
            cm_out_cos = _ap(COSV, 0, [[1, T], [T, NC2]])
            nc.scalar.activation(out=cm_out_sin, in_=ang, func=ACT.Sin)
            nc.scalar.activation(out=cm_out_cos, in_=ang, func=ACT.Abs)
            nc.scalar.activation(out=COSV[:], in_=COSV[:], func=ACT.Sin,
                                 bias=halfpi[:], scale=-1.0)

            S1 = _ap(SCR, 0, [[1, NT]])
            S2 = _ap(SCR, NT, [[1, NT]])
            PX = _ap(PXY, 0, [[1, NT]])
            PY = _ap(PXY, NT, [[1, NT]])

            nc.vector.tensor_mul(out=S1, in0=SAv, in1=cb(C_CADT))
            nc.vector.tensor_add(out=S1, in0=S1, in1=cb(C_P0X))
            nc.vector.tensor_mul(out=S2, in0=SEv, in1=cb(C_CEDT))
            nc.vector.tensor_sub(out=PX, in0=S1, in1=S2)
            nc.vector.tensor_mul(out=S1, in0=SAv, in1=cb(C_SADT))
            nc.vector.tensor_add(out=S1, in0=S1, in1=cb(C_P0Y))
            nc.vector.tensor_mul(out=S2, in0=SEv, in1=cb(C_SEDT))
            nc.vector.tensor_sub(out=PY, in0=S1, in1=S2)

            # ---------------- body-frame components ----------------
            # SINV-products first (COSV lands later on ACT).
            CE = _ap(COSV, 0, [[1, NT]])
            CA = _ap(COSV, NT, [[1, NT]])
            SE_ = _ap(SINV, 0, [[1, NT]])
            SA_ = _ap(SINV, NT, [[1, NT]])
            R12 = pool.tile([PB, 2 * NT], F32, tag="tST")
            R1X = _ap(R12, 0, [[1, NT]])
            R1Y = _ap(R12, NT, [[1, NT]])
            R34 = pool.tile([PB, 2 * NT], F32)
            R2X = _ap(R34, 0, [[1, NT]])
            R2Y = _ap(R34, NT, [[1, NT]])

            nc.vector.tensor_mul(out=R1X, in0=SE_, in1=PY)
            nc.vector.tensor_mul(out=R1Y, in0=SE_, in1=PX)
            nc.vector.tensor_mul(out=R2X, in0=SA_, in1=PY)
            nc.vector.tensor_mul(out=R2Y, in0=SA_, in1=PX)
            nc.vector.tensor_mul(out=S1, in0=CE, in1=PX)
            nc.vector.tensor_add(out=R1X, in0=R1X, in1=S1)   # rel1x
            nc.vector.tensor_mul(out=S2, in0=CE, in1=PY)
            nc.vector.tensor_sub(out=R1Y, in0=S2, in1=R1Y)   # rel1y
            nc.vector.tensor_mul(out=S1, in0=CA, in1=PX)
            nc.vector.tensor_add(out=R2X, in0=R2X, in1=S1)   # -rel2x; |.| ok
            nc.vector.tensor_mul(out=S2, in0=CA, in1=PY)
            nc.vector.tensor_sub(out=R2Y, in0=R2Y, in1=S2)   # rel2y

            # |rel| on ACT, then the shifted max-tree:
            # dist = max(max(|r1x|+d1, |r1y|) + d3, max(|r2x|+d2, |r2y|)) - k2y
            # with d1=k1y-k1x, d2=k2y-k2x, d3=k2y-k1y; -k2y lands after the
            # min-reduce as a [128,64] op (k's are constant over t).
            for R in (R1X, R1Y, R2X, R2Y):
                nc.scalar.activation(out=R, in_=R, func=ACT.Abs)
            nc.vector.tensor_add(out=R1X, in0=R1X, in1=cb(C_D1))
            nc.vector.tensor_tensor(out=R1X, in0=R1X, in1=R1Y, op=OP.max)
            nc.vector.tensor_add(out=R2X, in0=R2X, in1=cb(C_D2))
            nc.vector.tensor_tensor(out=R2X, in0=R2X, in1=R2Y, op=OP.max)
            nc.vector.tensor_add(out=R1X, in0=R1X, in1=cb(C_D3))
            nc.vector.tensor_tensor(out=R1X, in0=R1X, in1=R2X, op=OP.max)

            H = pool.tile([PB, A], F32)
            nc.vector.tensor_reduce(out=H[:],
                                    in_=_ap(R12, 0, [[T, A], [1, T]]),
                                    axis=mybir.AxisListType.X, op=OP.min)
            nc.vector.tensor_sub(out=H[:], in0=H[:], in1=c(C_K2Y))
            OUTT = pool.tile([PB, A], F32)
            nc.scalar.activation(out=H[:], in_=H[:], func=ACT.Tanh, scale=0.1)
            nc.vector.tensor_scalar_mul(out=OUTT[:], in0=H[:], scalar1=5.0)
            nc.sync.dma_start(out=out[:], in_=OUTT[:])

    nc.compile()
    return nc


def _get_nc(dt_uniform, k_red):
    key = ("nc", dt_uniform, k_red)
    if key not in _cache:
        _cache[key] = _build(dt_uniform, k_red)
    return _cache[key]


def _make_runner(nc):
    """One-time build of a cached jitted SPMD executable for nc (the
    equivalent of bass2jax.run_bass_via_pjrt, but reusable across calls so
    repeated kernel() invocations skip the jax retrace)."""
    import jax
    from jax.sharding import Mesh, PartitionSpec
    from jax.experimental.shard_map import shard_map
    from concourse import bass2jax, mybir as _mybir

    bass2jax.install_neuronx_cc_hook()
    partition_name = (nc.partition_id_tensor.name
                      if nc.partition_id_tensor else None)
    in_names, out_names, out_avals, zero_outs = [], [], [], []
    for alloc in nc.m.functions[0].allocations:
        if not isinstance(alloc, _mybir.MemoryLocationSet):
            continue
        name = alloc.memorylocations[0].name
        if alloc.kind == "ExternalInput":
            if name != partition_name:
                in_names.append(name)
        elif alloc.kind == "ExternalOutput":
            shape = tuple(alloc.tensor_shape)
            dtype = _mybir.dt.np(alloc.dtype)
            out_names.append(name)
            out_avals.append(jax.core.ShapedArray(shape, dtype))
            zero_outs.append(np.zeros(shape, dtype))
    n_params = len(in_names)
    all_names = in_names + out_names
    if partition_name is not None:
        all_names = all_names + [partition_name]
    donate = tuple(range(n_params, n_params + len(out_names)))

    def _body(*args):
        operands = list(args)
        if partition_name is not None:
            operands.append(bass2jax.partition_id_tensor())
        outs = bass2jax._bass_exec_p.bind(
            *operands, out_avals=tuple(out_avals), in_names=tuple(all_names),
            out_names=tuple(out_names), lowering_input_output_aliases=(),
            sim_require_finite=True, sim_require_nnan=True, nc=nc)
        return tuple(outs)

    mesh = Mesh(np.asarray(jax.devices()[:N_CORES]), ("core",))
    in_specs = (PartitionSpec("core"),) * (n_params + len(out_names))
    out_specs = (PartitionSpec("core"),) * len(out_names)
    sharded = jax.jit(
        shard_map(_body, mesh=mesh, in_specs=in_specs, out_specs=out_specs,
                  check_rep=False),
        donate_argnums=donate, keep_unused=True)
    concat_zeros = [np.zeros((N_CORES * z.shape[0], *z.shape[1:]), z.dtype)
                    for z in zero_outs]

    def run(full_data_2d):  # [B, A*F] -> [B, A]
        outs = sharded(full_data_2d, *[z.copy() for z in concat_zeros])
        return np.asarray(outs[out_names.index("out")])

    return run


def _params_for(data: np.ndarray):
    dt = data[..., 14]
    dt0 = float(dt.flat[0])
    dt_uniform = dt0 if bool(np.all(dt == dt0)) else None
    vmax = float(np.abs(data[..., [2, 6]]).max())
    # slots j >= k_red have |v_j| <= pi: while |v| > 2.2 each step shrinks
    # |v| by >= 9*dt_min*tanh(4.4), and the map keeps |v| <= pi once below
    # (valid when the max step 9*dt_max <= pi; otherwise reduce every slot).
    dt_min = float(dt.min())
    dt_max = float(dt.max())
    shrink = 9.0 * dt_min * 0.9997
    if 9.0 * dt_max > np.pi or shrink <= 1e-6:
        k_red = T
    else:
        k_red = int(min(T, max(0, np.ceil((vmax - np.pi) / shrink) + 1)))
    return dt_uniform, k_red


def _run(data: np.ndarray, trace: bool = False):
    data = np.ascontiguousarray(data, dtype=np.float32)
    assert data.shape == (B, A, F), data.shape
    dt_uniform, k_red = _params_for(data)
    nc = _get_nc(dt_uniform, k_red)
    in_maps = [{"data": data[c * PB:(c + 1) * PB].reshape(PB, A * F)}
               for c in range(N_CORES)]
    res = run_bass_kernel_spmd(nc, in_maps, core_ids=list(range(N_CORES)),
                               trace=trace)
    full = np.concatenate([res.results[c]["out"] for c in range(N_CORES)],
                          axis=0)
    return full, res


def kernel(data: np.ndarray) -> np.ndarray:
    data = np.ascontiguousarray(data, dtype=np.float32)
    assert data.shape == (B, A, F), data.shape
    dt_uniform, k_red = _params_for(data)
    key = ("runner", dt_uniform, k_red)
    if key not in _cache:
        _cache[key] = _make_runner(_get_nc(dt_uniform, k_red))
    return _cache[key](data.reshape(B, A * F)).astype(np.float32)
